# revision 2
# baseline (speedup 1.0000x reference)
"""Trainium2 Bass kernel: per-row top-50 stats over [4096, 16384] f32.

Baseline structure (exact two-pass top-k) with the candidate-position ->
global-index resolution done by a pair of GPSIMD local_scatter ops instead of
the 50-instruction DVE select-gather (which was ~36% of DVE busy time):

  1. Per-chunk top-8 (chunk=256, 64 chunks) via DVE Max8 -> 512 candidates.
  2. Per-chunk positions via DVE MaxIndex; global index = chunkbase + pos.
  3. 7 rounds of Max8/MaxIndex/MatchReplace on the 512-wide candidate array
     -> top-56 values + candidate slots, value-descending; ties (equal f32
     values, which the RNG grid does produce) resolve to ascending index via
     MaxIndex first-match semantics, matching lax.top_k.
  4. Slot -> global index resolved with an inverse-permutation double
     scatter on GPSIMD: tmp[slot[r]] = r+1, then out[tmp[k]-1] = gidx[k]
     (negative indices ignored), i.e. out[r] = gidx[slot[r]].
  5. Stats computed on ACT; one [128, 54] store per tile.

Sharding: pure data parallel, 8 cores x 512 rows, 4 tiles of 128 rows.
"""

import sys

if "/opt/trn_rl_repo" not in sys.path:
    sys.path.insert(0, "/opt/trn_rl_repo")

import numpy as np

import concourse.bass as bass
import concourse.tile as tile
from concourse import bacc, mybir
from concourse.bass_utils import run_bass_kernel_spmd

P = 128              # partitions (rows per tile)
N = 16384            # row length
C = 256              # chunk size
NCH = N // C         # 64 chunks per row
CAND = NCH * 8       # 512 candidates per row
K = 50               # top-k reported
KR = 56              # 7 rounds x 8 extracted
NCORES = 8
ROWS_PER_CORE = 512
NT = ROWS_PER_CORE // P   # 4 tiles per core
OUTW = 4 + K         # 54 output columns
XSEG = 4096          # x is loaded in 4 column segments per tile
SENTINEL = -1e30

f32 = mybir.dt.float32
u32 = mybir.dt.uint32
u16 = mybir.dt.uint16
i16 = mybir.dt.int16

A = mybir.AluOpType

_CACHE = {}


def _build(num_devices=NCORES, nt=NT):
    key = ("nc", num_devices, nt)
    if key in _CACHE:
        return _CACHE[key]
    rows = nt * P
    nc = bacc.Bacc(
        "TRN2", target_bir_lowering=False, debug=False, num_devices=num_devices
    )
    x_d = nc.dram_tensor("inputs", [rows, N], f32, kind="ExternalInput").ap()
    o_d = nc.dram_tensor("out", [rows, OUTW], f32, kind="ExternalOutput").ap()

    with tile.TileContext(nc) as tc:
        with (
            tc.tile_pool(name="xp", bufs=8) as xp,
            tc.tile_pool(name="cand", bufs=2) as cp,
            tc.tile_pool(name="small", bufs=3) as sp,
            tc.tile_pool(name="const", bufs=1) as kp,
        ):
            # chunk base index of each candidate slot: (slot//8)*C
            chunkbase = kp.tile([P, CAND], u32)
            nc.gpsimd.iota(
                chunkbase[:], pattern=[[C, NCH], [0, 8]], base=0,
                channel_multiplier=0,
            )
            # rank+1 payload for the inverse-permutation scatter
            rankp1 = kp.tile([P, KR], i16)
            nc.gpsimd.iota(rankp1[:], pattern=[[1, KR]], base=1,
                           channel_multiplier=0)
            ones = kp.tile([P, CAND], i16)
            nc.gpsimd.memset(ones[:], 1)

            # software pipeline: emit tile t+1's stage-1 (DMA + Max8 +
            # MaxIndex) before tile t's stage-2 rounds, so the in-order DVE
            # queue fills the dependency bubbles of the serial stage-2
            # chain with independent stage-1 work.
            pend = None
            for t in range(nt):
                st1 = _emit_stage1(nc, t, xp, cp, chunkbase, x_d)
                if pend is not None:
                    _emit_stage2(nc, *pend, sp, cp, rankp1, ones, o_d)
                pend = st1
            _emit_stage2(nc, *pend, sp, cp, rankp1, ones, o_d)
    nc.compile()
    _CACHE[key] = nc
    return nc


def _emit_stage1(nc, t, xp, cp, chunkbase, x_d):
    xsegs = []
    for s in range(N // XSEG):
        xs = xp.tile([P, XSEG], f32, tag="x")
        nc.sync.dma_start(
            out=xs[:],
            in_=x_d[t * P:(t + 1) * P, s * XSEG:(s + 1) * XSEG],
        )
        xsegs.append(xs)

    cpseg = XSEG // C  # chunks per segment
    V = cp.tile([P, CAND], f32, tag="V")
    L = cp.tile([P, CAND], u32, tag="L")
    for c in range(NCH):
        xs = xsegs[c // cpseg]
        lo = (c % cpseg) * C
        nc.vector.max(
            out=V[:, c * 8:(c + 1) * 8], in_=xs[:, lo:lo + C]
        )
    for c in range(NCH):
        xs = xsegs[c // cpseg]
        lo = (c % cpseg) * C
        nc.vector.max_index(
            out=L[:, c * 8:(c + 1) * 8],
            in_max=V[:, c * 8:(c + 1) * 8],
            in_values=xs[:, lo:lo + C],
        )

    # global candidate indices as u16 (Pool)
    Iu = cp.tile([P, CAND], u32, tag="Iu")
    nc.gpsimd.tensor_tensor(
        out=Iu[:], in0=L[:], in1=chunkbase[:], op=A.add,
    )
    gix16 = cp.tile([P, CAND], u16, tag="gix16")
    nc.gpsimd.tensor_copy(out=gix16[:], in_=Iu[:])
    return t, V, gix16


def _emit_stage2(nc, t, V, gix16, sp, cp, rankp1, ones, o_d):
    # stage 2: top-56 of the candidates
    vals = sp.tile([P, KR], f32, tag="vals")
    pos16 = sp.tile([P, KR], u16, tag="pos16")
    Vw = cp.tile([P, CAND], f32, tag="Vw")
    src = V
    for r in range(7):
        nc.vector.max(out=vals[:, r * 8:(r + 1) * 8], in_=src[:])
        nc.vector.max_index(
            out=pos16[:, r * 8:(r + 1) * 8],
            in_max=vals[:, r * 8:(r + 1) * 8],
            in_values=src[:],
        )
        if r < 6:
            nc.vector.match_replace(
                out=Vw[:],
                in_to_replace=vals[:, r * 8:(r + 1) * 8],
                in_values=src[:],
                imm_value=SENTINEL,
            )
            src = Vw

    # slot -> global index via inverse-permutation double scatter (GPSIMD)
    tmp = cp.tile([P, CAND], i16, tag="tmp")
    nc.gpsimd.local_scatter(
        out_ap=tmp[:], data_ap=rankp1[:], idxs_ap=pos16[:].bitcast(i16),
        channels=P, num_elems=CAND, num_idxs=KR,
    )
    idxs2 = cp.tile([P, CAND], i16, tag="idxs2")
    nc.vector.tensor_tensor(out=idxs2[:], in0=tmp[:], in1=ones[:],
                            op=A.subtract)
    idx16 = sp.tile([P, KR], i16, tag="idx16")
    nc.gpsimd.local_scatter(
        out_ap=idx16[:], data_ap=gix16[:].bitcast(i16), idxs_ap=idxs2[:],
        channels=P, num_elems=KR, num_idxs=CAND,
    )
    idxf = sp.tile([P, KR], f32, tag="idxf")
    nc.gpsimd.tensor_copy(out=idxf[:], in_=idx16[:])

    # stats on ACT
    ot = sp.tile([P, OUTW], f32, tag="ot")
    s2 = sp.tile([P, 2], f32, tag="s2")
    d10 = sp.tile([P, 10], f32, tag="d10")
    nc.scalar.activation(
        out=d10[:], in_=idxf[:, :10],
        func=mybir.ActivationFunctionType.Copy,
        accum_out=s2[:, 0:1],
    )
    nc.scalar.activation(
        out=ot[:, 0:1], in_=s2[:, 0:1],
        func=mybir.ActivationFunctionType.Copy, scale=0.1,
    )
    nc.scalar.activation(
        out=d10[:], in_=vals[:, :10],
        func=mybir.ActivationFunctionType.Square,
        accum_out=s2[:, 1:2],
    )
    nc.scalar.activation(
        out=ot[:, 1:2], in_=s2[:, 1:2],
        func=mybir.ActivationFunctionType.Sqrt, scale=0.1,
    )
    nc.scalar.copy(out=ot[:, 2:3], in_=idxf[:, 0:1])
    nc.scalar.activation(
        out=ot[:, 3:4], in_=vals[:, 0:1],
        func=mybir.ActivationFunctionType.Abs,
    )
    nc.scalar.copy(out=ot[:, 4:4 + K], in_=idxf[:, :K])
    nc.sync.dma_start(out=o_d[t * P:(t + 1) * P, :], in_=ot[:])


def _run(inputs_np, **spmd_kwargs):
    nc = _build()
    in_maps = [
        {"inputs": inputs_np[i * ROWS_PER_CORE:(i + 1) * ROWS_PER_CORE]}
        for i in range(NCORES)
    ]
    res = run_bass_kernel_spmd(nc, in_maps, list(range(NCORES)), **spmd_kwargs)
    out = np.concatenate([r["out"] for r in res.results], axis=0)
    return out, res


def kernel(inputs):
    inputs_np = np.ascontiguousarray(np.asarray(inputs, dtype=np.float32))
    assert inputs_np.shape == (NCORES * ROWS_PER_CORE, N)
    out, _ = _run(inputs_np)
    return out
